# revision 13
# baseline (speedup 1.0000x reference)
"""DiagBlockAttention Trainium2 kernel v2 (Bass/Tile, 8 NeuronCores).

Problem (hardcoded from spec nn_DiagBlockAttention):
  x[16, 3136, 768] -> qkv = x @ w_qkv -> 12 heads x 64
  block-local attention over 4x4 spatial blocks (16 tokens each),
  softmax over the 16 tokens of each block per head
  out = attn_out @ w_out + b_out

Sharding: data-parallel over batch, 2 batches per core.

Design (990us baseline -> 529us measured; PE-matmul union ~92% of span,
HAM warm throughout):
- ALL matmuls bf16 (rel err ~5e-3 vs 2e-2 gate): overlappable LDWEIGHTS
  (f32r pays a fused serial weight load), 2x DVE rates, half the DMA.
- x is block-permuted AND transposed to d-major ON THE HOST, so the
  stage-A PE transposes vanish; x^T DMAs straight into SBUF.
- Token stream regrouped: per core 392 blocks -> 7 superchunks x 896
  tokens; each superchunk = 7 groups x 128 tokens (8 blocks). All
  attention matmuls use full 128 partitions and 128-col stationaries.
- Scores emitted parity-inner: consecutive matmuls hit disjoint PE row
  groups (even/odd head at rows 0:64/64:128), so pairs run concurrently
  on the 32x32 sub-arrays and weight loads overlap in-flight matmuls.
- PV matmul is swapped (stationary = v, moving = P^T) so attention
  output lands d-major (no stage-E transposes); odd heads land at PSUM
  partitions 64:128 via the tile_position col-group derived from
  out.base_partition()=64.
- Softmax sums via 64-col ones-stationary matmuls: the denominators
  arrive replicated across PSUM partitions 0:64/64:128, so 1/sums is a
  single full-width DVE reciprocal_approx_fast (the microcoded
  reciprocal() costs 3us per tile, 5x more) and one tensor_mul
  normalizes o^T in place. No partition broadcast is ever needed
  (VectorE/ScalarE are 128-lane lockstep; gpsimd cross-partition ops
  are ~us-slow).
- Software pipeline: scores run one unit ahead of PV; the first two
  units' exp->mask chains hide under the v projection; span0's four
  out-projection groups interleave into span1's pipeline and the last
  three migrate into the NEXT superchunk's qk-projection phase, so the
  PE stream stays dense across superchunk boundaries.
- Startup: weights stream in j-chunks ordered by first use across the
  three DMA-issuing queues (sync/scalar/gpsimd); sc0's x^T splits over
  six HW DMA engines; the cold-clock f32 bias-replication matmuls are
  deferred behind sc0's qk-projection.
- Output stored bf16 (host upcasts); bias add doubles as the
  psum->SBUF copy.
"""
import numpy as np
import ml_dtypes
from contextlib import ExitStack

import concourse.bass as bass
import concourse.mybir as mybir
import concourse.tile as tile
from concourse import bacc
from concourse.bass_utils import run_bass_kernel_spmd

# ---- problem constants ----
B, N, DIM = 16, 3136, 768
H, DH = 12, 64
J3 = 3 * H * DH              # 2304
SCALE = DH ** -0.5           # 0.125
NCORES = 8
B_LOC = B // NCORES          # 2
NTOK = B_LOC * N             # 6272 tokens per core
NSC = 7                      # superchunks per core
SC = NTOK // NSC             # 896 tokens per superchunk
NG = SC // 128               # 7 groups of 128 tokens (8 blocks)
KT = DIM // 128              # 6 k-tiles
NHP = H // 2                 # 6 head pairs
# attention spans: groups 0..3 (512 cols) and 4..6 (384 cols)
SPANS = [(0, 4), (4, 3)]     # (first group, ngroups)
F32 = mybir.dt.float32
BF16 = mybir.dt.bfloat16
BFNP = ml_dtypes.bfloat16

_CACHE = {}


def _build():
    nc = bacc.Bacc("TRN2", target_bir_lowering=False, debug=False)

    # host-prepped inputs: x d-major bf16 per superchunk, weights bf16
    x_d = nc.dram_tensor("x", [NSC, DIM, SC], BF16, kind="ExternalInput")
    wqkv_d = nc.dram_tensor("w_qkv", [DIM, J3], BF16, kind="ExternalInput")
    wout_d = nc.dram_tensor("w_out", [DIM, DIM], BF16, kind="ExternalInput")
    bout_d = nc.dram_tensor("b_out", [DIM], BF16, kind="ExternalInput")
    # output token-major (block order); host un-permutes
    o_d = nc.dram_tensor("o", [NSC, SC, DIM], BF16, kind="ExternalOutput")

    with tile.TileContext(nc) as tc, ExitStack() as ctx:
        const = ctx.enter_context(tc.tile_pool(name="const", bufs=1))
        wpool = ctx.enter_context(tc.tile_pool(name="w", bufs=1))
        xin = ctx.enter_context(tc.tile_pool(name="xin", bufs=2))
        qkp_ = ctx.enter_context(tc.tile_pool(name="qkp", bufs=2))
        vap = ctx.enter_context(tc.tile_pool(name="vap", bufs=2))
        otp = ctx.enter_context(tc.tile_pool(name="otp", bufs=2))
        mid = ctx.enter_context(tc.tile_pool(name="mid", bufs=4))
        outp = ctx.enter_context(tc.tile_pool(name="outp", bufs=3))

        ps_proj = ctx.enter_context(tc.tile_pool(name="ps_proj", bufs=2, space="PSUM"))
        ps_s = ctx.enter_context(tc.tile_pool(name="ps_s", bufs=4, space="PSUM"))
        ps_pv = ctx.enter_context(tc.tile_pool(name="ps_pv", bufs=2, space="PSUM"))

        # ---- input DMAs first (before any preamble compute), so the
        # HBM-bandwidth-bound 7.2MB weight+x load starts immediately.
        # Weights stream in j-chunks ordered by first use: qk-proj only
        # needs w[:, :, 0:128] to start.
        w_sb = wpool.tile([128, KT, J3], BF16)
        wo_sb = wpool.tile([128, KT, DIM], BF16)
        wq_src = wqkv_d.ap().rearrange("(kt p) j -> p kt j", p=128)
        wo_src = wout_d.ap().rearrange("(kt p) j -> p kt j", p=128)
        dma_engs = [nc.sync, nc.scalar, nc.gpsimd]

        def load_xT(sc):
            t = xin.tile([128, KT, SC], BF16, tag="xT")
            src = x_d.ap()[sc].rearrange("(kt p) t -> p kt t", p=128)
            if sc == 0:
                # startup-critical: six HW DMA engines move it in parallel
                for kt in range(KT):
                    dma_engs[kt % 3].dma_start(t[:, kt, :], src[:, kt, :])
            else:
                # steady-state prefetch: keep the scalar queue (exp/copies)
                # free of DMA triggers
                nc.sync.dma_start(t[:, 0:3, :], src[:, 0:3, :])
                nc.gpsimd.dma_start(t[:, 3:6, :], src[:, 3:6, :])
            return t

        # priority order: bias (tiny) and the first w chunk land on queues
        # not carrying x, so sc0's first matmuls start ~12us in; the rest
        # of the 7.2MB streams behind, always ahead of its consumption
        bias1 = const.tile([1, DIM], BF16)
        nc.scalar.dma_start(bias1[:], bout_d.ap().unsqueeze(0))
        nc.scalar.dma_start(w_sb[:, :, 0:256], wq_src[:, :, 0:256])
        xT_next = load_xT(0)
        for i, j0 in enumerate(range(256, J3, 256)):
            dma_engs[i % 3].dma_start(
                w_sb[:, :, j0:j0 + 256], wq_src[:, :, j0:j0 + 256])
        for i, j0 in enumerate(range(0, DIM, 384)):
            dma_engs[(i + 1) % 3].dma_start(
                wo_sb[:, :, j0:j0 + 384], wo_src[:, :, j0:j0 + 384])

        # ---- constants ----
        # 0/1 block-diag-16 mask, one [128,128] pattern repeated 4x in free
        mask = const.tile([128, 512], BF16)
        nc.gpsimd.memset(mask[:], 1.0)
        mask_v = mask[:].rearrange("p (g b i) -> p g b i", g=4, b=8)
        nc.gpsimd.affine_select(
            out=mask_v, in_=mask_v, compare_op=mybir.AluOpType.is_ge,
            fill=0.0, base=0, pattern=[[0, 4], [-16, 8], [0, 16]],
            channel_multiplier=1)
        nc.gpsimd.affine_select(
            out=mask_v, in_=mask_v, compare_op=mybir.AluOpType.is_ge,
            fill=0.0, base=15, pattern=[[0, 4], [16, 8], [0, 16]],
            channel_multiplier=-1)

        # 64 columns of ones: the sums matmul replicates the softmax
        # denominators across 64 PSUM partitions (same PE cost — the moving
        # stream is what's paid for), making the downstream reciprocal a
        # full-width DVE op with no partition broadcast needed.
        ones64 = const.tile([128, 64], BF16)
        nc.vector.memset(ones64[:], 1.0)

        # K=1 ones stationary: replicates the bias across the out-proj psum
        # partitions as the accumulation preload (bf16 so it streams at
        # 1 col/cycle)
        ones1 = const.tile([1, 128], BF16)
        nc.vector.memset(ones1[:], 1.0)

        pending_out = []
        for sc in range(NSC):
            # ---- A: x^T (prefetched one superchunk ahead) ----
            xT = xT_next
            if sc + 1 < NSC:
                xT_next = load_xT(sc + 1)

            # ---- B: q/k projection, d-major [j, t], with the previous
            # superchunk's remaining out-projection groups interleaved ----
            qk = qkp_.tile([128, H, SC], BF16, tag="qk")
            for jt in range(H):
                if pending_out and jt in (4, 7, 10):
                    pending_out.pop(0)()
                for half in range(2):
                    ts = slice(half * 448, (half + 1) * 448)
                    qp = ps_proj.tile([128, 448], F32, tag="ps_proj")
                    for kt in range(KT):
                        nc.tensor.matmul(
                            qp[:], w_sb[:, kt, jt * 128:(jt + 1) * 128],
                            xT[:, kt, ts],
                            start=(kt == 0), stop=(kt == KT - 1))
                    if (2 * jt + half) % 2 == 0:
                        nc.vector.tensor_copy(qk[:, jt, ts], qp[:])
                    else:
                        nc.scalar.copy(qk[:, jt, ts], qp[:])

            va = [vap.tile([128, NG, NHP, DH], BF16, tag=f"va{i}", name=f"va{i}")
                  for i in range(2)]
            oT = otp.tile([128, KT, SC], BF16, tag="oT")

            def emit_v_chunk(g, half):
                # v projection, token-major, split by head parity:
                # va0[tk, g, hp, dh] = v of head 2hp; va1 = head 2hp+1.
                # Evacuation on gpsimd: keeps the vector/scalar queues free
                # for the attention chain (exp/mask/recip/normalize), whose
                # in-order position behind copies was stalling the PE.
                vp = ps_proj.tile([128, 384], F32, tag="ps_proj")
                for kt in range(KT):
                    nc.tensor.matmul(
                        vp[:], xT[:, kt, g * 128:(g + 1) * 128],
                        w_sb[:, kt, 1536 + half * 384:1536 + (half + 1) * 384],
                        start=(kt == 0), stop=(kt == KT - 1))
                vv = vp[:].rearrange("p (hp b d) -> p hp b d", hp=3, b=2)
                hs = slice(3 * half, 3 * half + 3)
                nc.vector.tensor_copy(va[0][:, g, hs, :], vv[:, :, 0, :])
                nc.scalar.copy(va[1][:, g, hs, :], vv[:, :, 1, :])

            def emit_scores(hp, span):
                g0, ng = span
                T = ng * 128
                sp = [ps_s.tile([128, T], F32, tag="ps_s", name=f"sp{par}")
                      for par in range(2)]
                # parity-inner: consecutive matmuls hit disjoint PE row
                # groups (0:64 / 64:128), so pairs run concurrently and each
                # LDWEIGHTS overlaps the other parity's in-flight matmul
                for g in range(g0, g0 + ng):
                    gs = slice(g * 128, (g + 1) * 128)
                    ls = slice((g - g0) * 128, (g - g0 + 1) * 128)
                    for par in range(2):
                        rows = slice(64 * par, 64 * par + 64)
                        nc.tensor.matmul(sp[par][:, ls], qk[rows, 6 + hp, gs],
                                         qk[rows, hp, gs], start=True, stop=True)
                pm = []
                for par in range(2):
                    pe_t = mid.tile([128, T], BF16, tag="pexp", name=f"pe{par}")
                    nc.scalar.activation(pe_t[:], sp[par][:],
                                         mybir.ActivationFunctionType.Exp,
                                         scale=SCALE)
                    pmt = mid.tile([128, T], BF16, tag="pm", name=f"pm{par}")
                    nc.vector.tensor_mul(pmt[:], pe_t[:], mask[:, 0:T])
                    pm.append(pmt)
                return pm

            def emit_pv(hp, span, pm):
                g0, ng = span
                # ss lives in the pv pool so ps_s holds only score pairs:
                # 4 banks = true 2-unit scores lookahead for the exp/mask
                # pipeline.
                ss = ps_pv.tile([128, T := ng * 128], F32, tag="ps_pv", name="ss")
                po = ps_pv.tile([128, T], F32, tag="ps_pv", name="po")
                # sums first: the reciprocal runs on DVE while the PE still
                # streams the PV matmuls, shortening the normalize chain.
                # The ones64 stationary replicates the denominators across
                # partitions 0:64 / 64:128 so no partition broadcast is
                # needed downstream.
                nc.tensor.matmul(ss[0:64, :], ones64[:], pm[0][:],
                                 start=True, stop=True)
                nc.tensor.matmul(ss[64:128, :], ones64[:], pm[1][:],
                                 start=True, stop=True)
                rT = mid.tile([128, T], F32, tag="rT")
                nc.vector.reciprocal_approx_fast(rT[:], ss[:])
                for g in range(g0, g0 + ng):
                    ls = slice((g - g0) * 128, (g - g0 + 1) * 128)
                    nc.tensor.matmul(po[0:64, ls], va[0][:, g, hp, :],
                                     pm[0][:, ls], start=True, stop=True)
                    nc.tensor.matmul(po[64:128, ls], va[1][:, g, hp, :],
                                     pm[1][:, ls], start=True, stop=True)
                nc.vector.tensor_mul(oT[:, hp, g0 * 128:g0 * 128 + T], po[:], rT[:])

            # ---- E (interleaved): out projection + bias, store ----
            def emit_out(g, oT=oT, sc=sc):
                gs = slice(g * 128, (g + 1) * 128)
                ob = outp.tile([128, DIM], BF16, tag="out_sb")
                for half in range(2):
                    js = slice(half * 384, (half + 1) * 384)
                    op = ps_proj.tile([128, 384], F32, tag="ps_proj")
                    # bias rides the accumulation as a K=1 ones-outer-product
                    # preload, so evacuation is a plain copy on scalar and the
                    # vector queue carries only attention-chain ops
                    nc.tensor.matmul(op[:], ones1[:], bias1[:, js],
                                     start=True, stop=False)
                    for kt in range(KT):
                        nc.tensor.matmul(op[:], oT[:, kt, gs], wo_sb[:, kt, js],
                                         start=False, stop=(kt == KT - 1))
                    nc.scalar.copy(ob[:, js], op[:])
                nc.sync.dma_start(o_d.ap()[sc, gs], ob[:])

            # Pipeline: scores run two units ahead of pv (ps_s holds 2 sp
            # pairs); the v-projection is woven between attention units in
            # (g,half) chunks so every chain-latency stall slot is filled
            # with projection matmuls, and span0's out-projections interleave
            # into span1's pipeline the same way.
            units = [(hp, span) for span in SPANS for hp in range(NHP)]
            # v chunk (g,half) fills va[*][:,g,3*half:3*half+3]; pv of unit
            # (hp,span) reads half=hp//3 chunks for the span's groups, so
            # each iter's pre-list lands the chunks its own pv needs and the
            # post-list spreads the rest as stall filler.
            vpre = {2: [(2, 0), (3, 0)]}
            vpost = {0: [(0, 0), (1, 0)], 2: [(0, 1)], 3: [(1, 1), (2, 1)],
                     4: [(3, 1)], 5: [(4, 0), (5, 0)], 6: [(6, 0)],
                     7: [(4, 1), (5, 1)], 8: [(6, 1)]}
            scored = [emit_scores(*units[0])]
            for c in vpost[0]:
                emit_v_chunk(*c)
            scored.append(emit_scores(*units[1]))
            for u in range(2, len(units)):
                for c in vpre.get(u, []):
                    emit_v_chunk(*c)
                emit_pv(units[u - 2][0], units[u - 2][1], scored[u - 2])
                for c in vpost.get(u, []):
                    emit_v_chunk(*c)
                if NHP + 2 <= u <= NHP + 5:
                    emit_out(u - NHP - 2)
                scored.append(emit_scores(*units[u]))
            emit_pv(units[-2][0], units[-2][1], scored[-2])
            emit_pv(units[-1][0], units[-1][1], scored[-1])
            # groups 4..6 migrate into the next superchunk's qk-proj phase:
            # their chains (and the last pvs' normalize) drain while the PE
            # streams dense projection matmuls
            if sc + 1 < NSC:
                pending_out = [lambda g=g, f=emit_out: f(g) for g in range(4, NG)]
            else:
                for g in range(4, NG):
                    emit_out(g)

    nc.compile()
    return nc


def _to_stream(x):
    """[B_LOC, 3136, d] raster -> [NTOK, d] block-major stream."""
    b, n, d = x.shape
    x = x.reshape(b, 14, 4, 14, 4, d)          # b, br, ir, bc, ic, d
    x = x.transpose(0, 1, 3, 2, 4, 5)           # b, br, bc, ir, ic, d
    return x.reshape(b * n, d)


def _from_stream(o):
    """inverse of _to_stream: [NTOK, d] -> [B_LOC, 3136, d]."""
    d = o.shape[-1]
    o = o.reshape(B_LOC, 14, 14, 4, 4, d)       # b, br, bc, ir, ic, d
    o = o.transpose(0, 1, 3, 2, 4, 5)           # b, br, ir, bc, ic, d
    return o.reshape(B_LOC, N, d)


def _make_in_maps(x, w_qkv, w_out, b_out):
    x = np.ascontiguousarray(x, dtype=np.float32)
    wq = np.ascontiguousarray(w_qkv, dtype=np.float32).astype(BFNP)
    wo = np.ascontiguousarray(w_out, dtype=np.float32).astype(BFNP)
    bo = np.ascontiguousarray(b_out, dtype=np.float32).astype(BFNP)
    in_maps = []
    for c in range(NCORES):
        xs = _to_stream(x[c * B_LOC:(c + 1) * B_LOC])      # [6272, 768]
        xT = xs.reshape(NSC, SC, DIM).transpose(0, 2, 1)   # [7, 768, 896]
        xT = np.ascontiguousarray(xT).astype(BFNP)
        in_maps.append({"x": xT, "w_qkv": wq, "w_out": wo, "b_out": bo})
    return in_maps


def kernel(x, w_qkv, w_out, b_out):
    if "nc" not in _CACHE:
        _CACHE["nc"] = _build()
    nc = _CACHE["nc"]

    in_maps = _make_in_maps(x, w_qkv, w_out, b_out)
    res = run_bass_kernel_spmd(nc, in_maps, core_ids=list(range(NCORES)))
    out = np.concatenate(
        [_from_stream(res.results[c]["o"].reshape(NTOK, DIM))
         for c in range(NCORES)], axis=0)
    return out.astype(np.float32)

